# revision 33
# baseline (speedup 1.0000x reference)
"""Trainium2 Bass kernel for nn_FCGAT (fully-connected GAT variant).

Mathematical simplifications used (exact, not approximate):

1. The reference computes
   ``out = einsum('nkj,nkd->nkd', softmax(aa, axis=2), h)`` which is
   ``h[n,k,d] * sum_j softmax(aa)[n,k,j] == h[n,k,d]``.  The whole attention
   block (z tensor, aw1/ab1/aw2/ab2, softmax) is dead code in real
   arithmetic; only float rounding noise (~1e-14 rel) distinguishes it.
   The model reduces to, per step::

       h_s = lrelu(lrelu([towers | x_s] @ w1.T + b1) @ w2.T + b2)
       x_{s+1} = h_s + x_s

   followed by ``prod_k sigmoid(x_K @ ow[0] + ob[0])`` over the K nodes of
   each graph.

2. The residual is distributed through the (linear) first matmul of the
   next step: ``x_s = x_0 + sum_{t<s} h_t``, so

       mm1(s) = w1 @ [towers | x_0]  +  sum_{t<s} w1[:, DT:] @ h_t
       logits = ow @ x_0 + sum_t ow @ h_t

   both as PSUM accumulation groups.  x_s is never materialized and there
   are no elementwise adds at all.

Sharding: data-parallel over the batch dim N=128 -> 16 graphs (1024 rows)
per core across 8 NeuronCores; all weights replicated.

Written in raw Bass (explicit engine blocks + semaphores) rather than Tile:
this toolchain's walrus build allows only ONE sync-wait command per
instruction, and Tile's auto-generated synchronization routinely needs
several.  Raw Bass sidesteps it: standalone single-condition waits and
happens-before transitivity applied by construction.

Performance notes (the ACT engine is the serial bottleneck -- every h1/h2
eviction and the sigmoids run there, back-to-back; everything below is
about shortening that chain and the tails around it):
  * all matmul operands are bf16 (1 PE cycle/row, psum accumulation still
    fp32): data + weights converted on the host, h1/h2 written as bf16 by
    the ACT evictions.  This removes the float32r-rounding passes a fp32r
    kernel would need (DMA'd bf16 is directly matmul-legal) and halves DMA
    bytes.  The sigmoid/product path stays f32 (bf16 there measured 3-6e-2
    rel err, over the 2e-2 gate);
  * input rows are pre-transposed to feature-major [D1, R] on the host
    (free) so there are no on-device PE transposes and no identity matrix;
  * ACT-op cost is (free_size + const) regardless of partition count, so
    narrow intermediates are partition-stacked to halve their free size:
    - h2 (64 features) is computed by TWO mm2 matmuls into partition
      ranges [0:64]/[64:128] of one psum bank, so each h2 eviction is a
      [128, 256] act (398ns) instead of [64, 512] (612ns);
    - the logits are FOUR 256-column psum groups stacked two deep on
      partitions {0, 64} of two banks (psum matmul writes must base at
      0/32/64), so the two sigmoid evictions are [0:65, 256] acts whose
      gates align with the per-chunk last h2 evictions, and the product
      trees run on [0:65, 4, *] views with half the free size per level.
      Lanes 1..63 of this path compute garbage that is never read (the
      psum lanes are zeroed by DVE memsets so they stay finite);
  * CRITICAL HW CONSTRAINT found by bisection: matmuls whose stationary
    operand has 64 partitions at BASE 0 (walrus tile_position (0,0) with a
    64-row tile) pass the BIR verifier and the executing CoreSim but die
    with an opaque NRT INTERNAL error on real hardware.  All consumers of
    the h2 [0:64] half therefore contract over the full 128 partitions
    with ZERO-PADDED stationaries (rows 64:128 zero annihilate the
    mover's other half -- exact, and matmul cost depends only on output
    free size); the [64:128]-half consumers use base-64 stationaries,
    which work.  Every matmul keeps a hardware-validated tile config;
  * leaky_relu runs as Prelu (pwp `parametric_relu`, same alpha-slope
    function) which shares the `sigmoid_and_others` act-func-set with
    Sigmoid: ONE ~1.3us table load (prewarmed at t=0 on a const cell)
    instead of three;
  * the 4 data DMAs are split across the SP and ACT HWDGE queues; the
    const pack rides ahead of the data on SP (w1t gates the first matmul);
    the prewarm holds the ACT SEQ only ~40ns so the ACT-queue DMAs issue
    immediately behind it;
  * the x0-part matmul of each accumulation group issues before the h2
    wait (only the h2-term matmuls wait), hiding it under the eviction;
  * the two per-bank trees merge once they reach [*, 4, 8] (the results
    are placed adjacently) so the last 3 levels run once over all 8
    graph-columns; one partition-strided DMA ships both product rows.

On-chip layout is feature-major ([feature, row] on partitions) so the
linear layers contract over the partition dim on the tensor engine.
(Gating the output DMA a few tree levels early to overlap its descriptor
generation is structurally timing-safe but trips the CoreSim race
detector -- sync-edge ordering is the contract -- so it is not done.)
"""

from contextlib import ExitStack

import ml_dtypes
import numpy as np

import concourse.bass as bass
import concourse.mybir as mybir
from concourse.bass_utils import run_bass_kernel_spmd

N_CORES = 8
N, K, DT, D2 = 128, 64, 64, 64
D1 = DT + D2                # 128: [towers | x] feature dim
G = N // N_CORES            # 16 graphs per core
R = G * K                   # 1024 rows per core
CHUNK = 512                 # psum-bank limit on matmul moving free dim
NCHUNK = R // CHUNK
HCH = CHUNK // 2            # 256: logits column granularity
NDATA_DMA = 4               # data DMAs: chunk halves, 2 per queue
DCOLS = R // NDATA_DMA      # 256 columns per data DMA

# packed constants layout (columns of a [128, CW] bf16 array)
C_W1 = 0                    # 0:128    w1.T (pre-transposed [d, o])
C_W2 = 128                  # 128:192  w2.T ([128, 64])
C_W1L2 = 192                # 192:320  [w1.T[64:128] | 0] (rows 0:64 real,
#                             64:128 zero: full-128 contraction so the
#                             [0:64]-half h2 movers use a vanilla tile)
C_B1 = 320                  # b1 column
C_B2 = 321                  # b2 stacked twice (rows 0:64 and 64:128)
C_OW = 322                  # ow[0] in rows 64:128
C_OW2 = 323                 # [ow[0] | 0]: rows 0:64 real, 64:128 zero
C_OB = 324                  # ob[0] in rows 0 and 64
CW = 325

_F32 = mybir.dt.float32
_BF16 = mybir.dt.bfloat16

# Results of the last hardware run (for the local test harness; the grading
# path only uses the return value of kernel()).
LAST_RESULT = None

_PROGRAM_CACHE = {}


def _build_program(kk: int, act_fn=None) -> bass.Bass:
    # act_fn override: the executing CoreSim implements Relu but not
    # Prelu; HW runs Prelu (same function, shared table set with Sigmoid)
    LRELU = act_fn or mybir.ActivationFunctionType.Prelu
    SIGMOID = mybir.ActivationFunctionType.Sigmoid

    nc = bass.Bass()
    const_d = nc.declare_dram_parameter("cpack", [128, CW], _BF16,
                                        isOutput=False)
    xct_d = nc.declare_dram_parameter("xct", [D1, R], _BF16, isOutput=False)
    # out[p, q*4+t] = product for graph q*8 + p*4 + t (host reorders)
    out_d = nc.declare_dram_parameter("out", [2, G // 2], _F32, isOutput=True)

    # ---- instruction numbering (semaphore values), computed up front ----
    # PE: per step s: NCHUNK mm1 groups of (1+s) matmuls, then NCHUNK
    # mm2s; finally the logits: 4 x0-part matmuls, 4 h2-part matmuls per
    # non-final step, then per bank q the final-step pair (stop).
    # The 4 + 4*(kk-1) "ready" logits matmuls (x0 parts + non-final-step
    # h2 parts) are woven into the last step's PE idle gaps -- 2 before
    # mm2(last,0), 4 more before mm2(last,1), the rest after -- so the
    # final-step pairs (which gate the sigmoids) issue with no backlog.
    assert NCHUNK == 2
    backlog = 4 + 4 * (kk - 1) if kk > 0 else 0
    weave = (min(2, backlog), min(4, backlog - min(2, backlog)))
    pe = 0
    pe_mm1 = {}
    pe_mm2 = {}
    pe_mm3 = {}
    for s in range(kk):
        for c in range(NCHUNK):
            pe += 1 + 2 * s
            pe_mm1[(s, c)] = pe
        last = s == kk - 1
        if last:
            pe += weave[0]
            pe += 2
            pe_mm2[(s, 0)] = pe
            pe += weave[1]
            pe += 2
            pe_mm2[(s, 1)] = pe
            pe += backlog - weave[0] - weave[1]
        else:
            for c in range(NCHUNK):
                pe += 2
                pe_mm2[(s, c)] = pe
    if kk == 0:
        pe += 4               # x0 parts (with stop)
        pe_mm3[0] = pe - 2
        pe_mm3[1] = pe
    else:
        for q in range(2):
            pe += 2
            pe_mm3[q] = pe

    # ACT: one table-prewarm dummy (sigmoid set, which also serves Prelu),
    # then per step s: h1(s,0..1), h2(s,0..1); finally sig(0..1)
    ACT0 = 1

    def act_h1(s, c):
        return ACT0 + 4 * s + c + 1

    def act_h2(s, c):
        return ACT0 + 4 * s + 2 + c + 1

    def act_sig(q):
        return ACT0 + 4 * kk + q + 1

    # DVE: two psum-lane memsets, then per logits bank a 3-level strided
    # multiply tree down to [*, 4, 8]; the two results are placed
    # adjacently so the last 3 levels run once over all 8 graph-columns
    dve_prod = 2 + 2 * 3 + 3

    with ExitStack() as ctx:
        cs = ctx.enter_context(nc.sbuf_tensor([128, CW], _BF16))
        xcT = ctx.enter_context(nc.sbuf_tensor([D1, R], _BF16))
        h1s = ctx.enter_context(
            nc.sbuf_tensor([D1, kk * NCHUNK * CHUNK + 1], _BF16))
        h2s = ctx.enter_context(
            nc.sbuf_tensor([D1, kk * NCHUNK * HCH + 1], _BF16))
        # sigmoid/product path in f32 (see module docstring); rows 0 and
        # 64 carry the two stacked logits rows, lanes 1..63 are garbage
        sig2 = ctx.enter_context(nc.sbuf_tensor([128, CHUNK], _F32))
        ptree = ctx.enter_context(nc.sbuf_tensor([128, CHUNK], _F32))
        prod = ctx.enter_context(nc.sbuf_tensor([128, G // 2], _F32))
        warm = ctx.enter_context(nc.sbuf_tensor([1, 2], _F32))
        # full-bank psum allocations; h-chunks rotate over two banks by
        # parity, the logits use two more banks (two stacked groups each)
        ps1 = [ctx.enter_context(nc.psum_tensor(f"ps1_{p}", [D1, 512], _F32))
               for p in range(2)]
        ps2 = [ctx.enter_context(nc.psum_tensor(f"ps2_{p}", [D1, 512], _F32))
               for p in range(2)]
        ps3 = [ctx.enter_context(nc.psum_tensor(f"ps3_{p}", [128, 512], _F32))
               for p in range(2)]
        sem_const = ctx.enter_context(nc.semaphore("sem_const"))
        sem_data = [ctx.enter_context(nc.semaphore(f"sem_d{j}"))
                    for j in range(NDATA_DMA)]
        sem_out = ctx.enter_context(nc.semaphore("sem_out"))
        pe_sem = ctx.enter_context(nc.semaphore("pe_sem"))
        act_sem = ctx.enter_context(nc.semaphore("act_sem"))
        dve_sem = ctx.enter_context(nc.semaphore("dve_sem"))
        block = ctx.enter_context(nc.Block())

        w1t = cs[:, C_W1:C_W1 + 128]
        w1t_lo = cs[DT:D1, C_W1:C_W1 + 128]      # base-64, [64:128] movers
        w1t_lo2f = cs[:, C_W1L2:C_W1L2 + 128]    # full-128, zero bottom
        w2t = cs[:, C_W2:C_W2 + 64]
        b1 = cs[:, C_B1:C_B1 + 1]
        b2st = cs[:, C_B2:C_B2 + 1]
        owc = cs[DT:D1, C_OW:C_OW + 1]
        owc2f = cs[:, C_OW2:C_OW2 + 1]           # full-128, zero bottom
        obc2 = cs[0:65, C_OB:C_OB + 1]

        def ps1_ap(c):
            return ps1[c % 2][:, 0:CHUNK]

        def ps2_ap(c):
            return ps2[c % 2][:, 0:CHUNK]

        def h1_ap(s, c):
            off = (s * NCHUNK + c) * CHUNK
            return h1s[:, off:off + CHUNK]

        def h2_blk(s, c):
            # [128, 256]: rows 0:64 = h2 of chunk rows 0:256,
            # rows 64:128 = h2 of chunk rows 256:512 (partition-stacked by
            # the mm2 pair so the eviction is a [128, 256] act).  Consumers
            # of the [0:64] half contract over all 128 partitions with a
            # zero-padded stationary (rows 64:128 of the mover are
            # annihilated) so every matmul keeps a probe-validated tile
            # config; the [64:128] half uses base-64 stationaries as in
            # the unstacked kernel.
            off = (s * NCHUNK + c) * HCH
            return h2s[:, off:off + HCH]

        def data_dma(eng, j):
            cols = slice(j * DCOLS, (j + 1) * DCOLS)
            eng.dma_start(
                xcT[:, cols], xct_d[:, cols]
            ).then_inc(sem_data[j], 16)

        @block.sync
        def _(sync):
            # consts ride first on the SP queue (w1t gates the first mm1),
            # then the odd data quarters; even quarters go on the ACT queue
            sync.dma_start(cs[:, :], const_d[:, :]).then_inc(sem_const, 16)
            data_dma(sync, 1)
            data_dma(sync, 3)
            # the products sit on partition rows 0 and 64 (engine lanes
            # are fixed); one partition-strided DMA ships both rows
            sync.wait_ge(dve_sem, dve_prod)
            sync.dma_start(
                out_d[:, :], prod[0:65:64, 0:G // 2]
            ).then_inc(sem_out, 16)
            sync.wait_ge(sem_out, 16)

        @block.tensor
        def _(tensor):
            wm = {}

            def twait(sem, val):
                # monotone watermark: skip waits already implied by an
                # earlier wait on the same semaphore
                if wm.get(id(sem), 0) < val:
                    wm[id(sem)] = val
                    tensor.wait_ge(sem, val)

            twait(sem_const, 16)
            for s in range(kk):
                for c in range(NCHUNK):
                    sl = slice(c * CHUNK, (c + 1) * CHUNK)
                    if s == 0:
                        # chunk c's columns arrive as DMAs 2c (ACT queue)
                        # and 2c+1 (SP queue)
                        twait(sem_data[2 * c], 16)
                        twait(sem_data[2 * c + 1], 16)
                    # psum-bank WAR: the previous user of this parity bank
                    # must have been evicted
                    if s >= 1:
                        twait(act_sem, act_h1(s - 1, c))
                    # x0-part issues before the h2 wait
                    nc.tensor.matmul(
                        ps1_ap(c), w1t, xcT[:, sl],
                        start=True, stop=(s == 0),
                    ).then_inc(pe_sem, 1)
                    for t in range(s):
                        if t == s - 1:
                            twait(act_sem, act_h2(s - 1, c))
                        nc.tensor.matmul(
                            ps1[c % 2][:, 0:HCH], w1t_lo2f, h2_blk(t, c),
                            start=False, stop=False,
                        ).then_inc(pe_sem, 1)
                        nc.tensor.matmul(
                            ps1[c % 2][:, HCH:CHUNK], w1t_lo,
                            h2_blk(t, c)[DT:D1, :],
                            start=False, stop=(t == s - 1),
                        ).then_inc(pe_sem, 1)
                # logits backlog: bank q, stacked partition 64p <-
                # logical columns (2q+p)*256 (bank q = graphs q*8..q*8+8).
                # All of these are data-ready by the last step; weave them
                # into the PE idle gaps around the last mm2s.
                def mm3_x0(q, p):
                    qsl = slice((2 * q + p) * HCH, (2 * q + p + 1) * HCH)
                    nc.tensor.matmul(
                        ps3[q][64 * p:64 * p + 1, 0:HCH], owc,
                        xcT[DT:D1, qsl],
                        start=True, stop=(kk == 0),
                    ).then_inc(pe_sem, 1)

                def mm3_h2(s, q, p, stop):
                    # h2 half p of chunk q holds the chunk's rows p*256..
                    nc.tensor.matmul(
                        ps3[q][64 * p:64 * p + 1, 0:HCH],
                        owc2f if p == 0 else owc,
                        h2_blk(s, q) if p == 0
                        else h2_blk(s, q)[DT:D1, :],
                        start=False, stop=stop,
                    ).then_inc(pe_sem, 1)

                last = s == kk - 1
                if last:
                    twait(dve_sem, 2)
                    queue = [(q, p) for q in range(2) for p in range(2)]
                    items = [("x0", None, q, p) for q, p in queue]
                    items += [("h2", t, q, p) for t in range(kk - 1)
                              for q, p in queue]

                    def emit(n):
                        for _ in range(n):
                            kind, t, q, p = items.pop(0)
                            if kind == "x0":
                                mm3_x0(q, p)
                            else:
                                mm3_h2(t, q, p, stop=False)

                    def mm2_pair(c):
                        # stack the chunk's h2 on 128 partitions: chunk
                        # rows 0:256 -> partitions 0:64, 256:512 -> 64:128
                        nc.tensor.matmul(
                            ps2[c % 2][0:DT, 0:HCH], w2t,
                            h1_ap(s, c)[:, 0:HCH],
                            start=True, stop=True,
                        ).then_inc(pe_sem, 1)
                        nc.tensor.matmul(
                            ps2[c % 2][DT:D1, 0:HCH], w2t,
                            h1_ap(s, c)[:, HCH:CHUNK],
                            start=True, stop=True,
                        ).then_inc(pe_sem, 1)

                    emit(weave[0])
                    twait(act_sem, act_h1(s, 0))
                    if s >= 1:
                        twait(act_sem, act_h2(s - 1, 0))
                    mm2_pair(0)
                    emit(weave[1])
                    twait(act_sem, act_h1(s, 1))
                    if s >= 1:
                        twait(act_sem, act_h2(s - 1, 1))
                    mm2_pair(1)
                    emit(len(items))
                else:
                    for c in range(NCHUNK):
                        twait(act_sem, act_h1(s, c))
                        if s >= 1:
                            twait(act_sem, act_h2(s - 1, c))
                        nc.tensor.matmul(
                            ps2[c % 2][0:DT, 0:HCH], w2t,
                            h1_ap(s, c)[:, 0:HCH],
                            start=True, stop=True,
                        ).then_inc(pe_sem, 1)
                        nc.tensor.matmul(
                            ps2[c % 2][DT:D1, 0:HCH], w2t,
                            h1_ap(s, c)[:, HCH:CHUNK],
                            start=True, stop=True,
                        ).then_inc(pe_sem, 1)
            if kk == 0:
                twait(dve_sem, 2)
                for j in range(NDATA_DMA):
                    twait(sem_data[j], 16)
                for q in range(2):
                    for p in range(2):
                        qsl = slice((2 * q + p) * HCH,
                                    (2 * q + p + 1) * HCH)
                        nc.tensor.matmul(
                            ps3[q][64 * p:64 * p + 1, 0:HCH], owc,
                            xcT[DT:D1, qsl],
                            start=True, stop=True,
                        ).then_inc(pe_sem, 1)
            else:
                # final-step pairs: the only logits matmuls that wait
                s = kk - 1
                for q in range(2):
                    twait(act_sem, act_h2(s, q))
                    for p in range(2):
                        nc.tensor.matmul(
                            ps3[q][64 * p:64 * p + 1, 0:HCH],
                            owc2f if p == 0 else owc,
                            h2_blk(s, q) if p == 0
                            else h2_blk(s, q)[DT:D1, :],
                            start=False, stop=True,
                        ).then_inc(pe_sem, 1)

        @block.scalar
        def _(scalar):
            # Prewarm the single shared ACT table (sigmoid_and_others serves
            # both Sigmoid and Prelu) immediately at t=0: the input is the
            # framework's preamble-memset const-0.0 cell, so no DMA wait is
            # needed and the load finishes well before the first eviction.
            # (Bias reads from `cs` are ordered behind the const DMA
            # transitively through each eviction's PE wait.)
            zcell = nc.const_aps.aps[(mybir.dt.float32, 0.0)][0:1, 0:1]
            nc.scalar.activation(
                warm[0:1, 0:1], zcell, SIGMOID
            ).then_inc(act_sem, 1)
            # even data quarters ride this engine's HWDGE queue, in parallel
            # with the SP queue.  The prewarm above holds the ACT SEQ only
            # ~40ns (the table load runs on the ACT engine in the
            # background), so these issue right away.
            data_dma(scalar, 0)
            data_dma(scalar, 2)
            seen = 0
            for s in range(kk):
                for c in range(NCHUNK):
                    if pe_mm1[(s, c)] > seen:
                        seen = pe_mm1[(s, c)]
                        scalar.wait_ge(pe_sem, seen)
                    nc.scalar.activation(
                        h1_ap(s, c), ps1_ap(c), LRELU,
                        bias=b1, alpha=0.01,
                    ).then_inc(act_sem, 1)
                for c in range(NCHUNK):
                    if pe_mm2[(s, c)] > seen:
                        seen = pe_mm2[(s, c)]
                        scalar.wait_ge(pe_sem, seen)
                    nc.scalar.activation(
                        h2_blk(s, c), ps2[c % 2][:, 0:HCH], LRELU,
                        bias=b2st, alpha=0.01,
                    ).then_inc(act_sem, 1)
            for q in range(2):
                if pe_mm3[q] > seen:
                    seen = pe_mm3[q]
                    scalar.wait_ge(pe_sem, seen)
                nc.scalar.activation(
                    sig2[0:65, q * HCH:(q + 1) * HCH],
                    ps3[q][0:65, 0:HCH],
                    SIGMOID, bias=obc2,
                ).then_inc(act_sem, 1)

        @block.vector
        def _(vector):
            # zero the logits psum banks first: the [0:65]-partition
            # sigmoid APs read lanes 1..63 that no matmul writes, and
            # uninitialized psum could hold NaN
            for q in range(2):
                nc.vector.memset(ps3[q][0:65, 0:HCH], 0.0).then_inc(
                    dve_sem, 1)
            # per-graph product: per bank q, 3 tree levels over
            # [0:65 partitions, 4 graphs, *] (real data in rows 0/64);
            # bank 0's levels run while bank 1's logits are in flight.
            # The [*, 4, 8] results land adjacently (cols 192:224 and
            # 224:256 of ptree) so the last 3 levels run ONCE over all
            # 8 graph-columns -- one DVE chain instead of two.
            GQ = G // 4
            dve_val = 2

            def gv(t, off, g, length):
                ap = t[0:65, off:off + g * length]
                return ap.rearrange("p (g j) -> p g j", g=g)

            def level(src_t, src_off, dst_t, dst_off, g, half, wait):
                nonlocal_dve = level_state
                if wait is not None:
                    # DVE completion is not implied by issue order;
                    # chained levels need an explicit completion wait
                    vector.wait_ge(dve_sem, wait)
                pv = gv(src_t, src_off, g, 2 * half)
                if dst_t is prod:
                    dst = prod[0:65, 0:G // 2].rearrange(
                        "p (g j) -> p g j", g=g)
                else:
                    dst = gv(dst_t, dst_off, g, half)
                nc.vector.tensor_tensor(
                    dst, pv[:, :, 0:half], pv[:, :, half:2 * half],
                    mybir.AluOpType.mult,
                ).then_inc(dve_sem, 1)
                nonlocal_dve[0] += 1

            level_state = [2]
            for q in range(2):
                vector.wait_ge(act_sem, act_sig(q))
                base = q * HCH
                level(sig2, q * HCH, ptree, base, GQ, 32, None)
                level(ptree, base, ptree, base + 128, GQ, 16,
                      level_state[0])
                level(ptree, base + 128, ptree, 192 + q * 32, GQ, 8,
                      level_state[0])
            # merged tail over [0:65, 8, *]
            level(ptree, 192, ptree, 448, 2 * GQ, 4, level_state[0])
            level(ptree, 448, ptree, 480, 2 * GQ, 2, level_state[0])
            level(ptree, 480, prod, 0, 2 * GQ, 1, level_state[0])

    return nc


def _pack_consts(w1, b1, w2, b2, ow, ob):
    cp = np.zeros((128, CW), ml_dtypes.bfloat16)
    # weights pre-transposed on host; bf16 conversion also on host
    cp[:, C_W1:C_W1 + 128] = w1.T.astype(ml_dtypes.bfloat16)
    cp[:, C_W2:C_W2 + 64] = w2.T.astype(ml_dtypes.bfloat16)
    cp[0:DT, C_W1L2:C_W1L2 + 128] = w1.T[DT:D1].astype(ml_dtypes.bfloat16)
    cp[:, C_B1] = b1.astype(ml_dtypes.bfloat16)
    b2b = b2.astype(ml_dtypes.bfloat16)
    cp[0:DT, C_B2] = b2b
    cp[DT:D1, C_B2] = b2b
    owb = ow.reshape(D2).astype(ml_dtypes.bfloat16)
    cp[DT:D1, C_OW] = owb
    cp[0:DT, C_OW2] = owb
    obb = np.asarray(ob, np.float32).reshape(()).astype(ml_dtypes.bfloat16)
    cp[0, C_OB] = obb
    cp[64, C_OB] = obb
    return cp


def _make_in_maps(towers, x, w1, b1, w2, b2, ow, ob):
    towers = np.asarray(towers, np.float32)
    x = np.asarray(x, np.float32)
    cpack = _pack_consts(
        np.asarray(w1, np.float32), np.asarray(b1, np.float32),
        np.asarray(w2, np.float32), np.asarray(b2, np.float32),
        np.asarray(ow, np.float32), np.asarray(ob, np.float32),
    )
    xc0 = np.concatenate(
        [towers.reshape(N * K, DT), x.reshape(N * K, D2)], axis=1
    ).astype(ml_dtypes.bfloat16)
    in_maps = []
    for i in range(N_CORES):
        sl = slice(i * R, (i + 1) * R)
        in_maps.append({
            "cpack": cpack,
            "xct": np.ascontiguousarray(xc0[sl].T),
        })
    return in_maps


def kernel(towers, x, w1, b1, w2, b2, aw1, ab1, aw2, ab2, ow, ob, k):
    global LAST_RESULT
    kk = int(k)

    if kk not in _PROGRAM_CACHE:
        _PROGRAM_CACHE[kk] = _build_program(kk)
    nc = _PROGRAM_CACHE[kk]

    in_maps = _make_in_maps(towers, x, w1, b1, w2, b2, ow, ob)
    res = run_bass_kernel_spmd(nc, in_maps, list(range(N_CORES)))
    LAST_RESULT = res
    out = np.concatenate([
        np.asarray(res.results[i]["out"]).reshape(2, G // 2)[p,
                                                             q * 4:(q + 1) * 4]
        for i in range(N_CORES)
        for q in range(2)
        for p in range(2)
    ])
    return out.astype(np.float32)
